# revision 1
# baseline (speedup 1.0000x reference)
"""Fused sp2norm-MHA kernel for Trainium2, 8 NeuronCores.

Model (per reference):
    qkv = x @ W_qkv.T ; split heads (H=16, hs=64)
    s = (q @ k.T) / sqrt(hs);  w = softplus(s) causal-masked
    out_h = (w @ v) / ||w||_row ;  out = concat(out_h) @ W_proj.T + b_proj

Sharding: core c = (b, g) with b = c // 4 (batch), g = c % 4 (head group of 4).
Each core computes its batch's QKV for its 4 heads, the attention, and a
partial projection over its 256 feature channels. The host sums the 4 partial
projections per batch and adds the bias (the unshard step).

On-device layout (per core):
    xT  [1024, 2048]  bf16  = x[b].T                  (c on partitions)
    Sᵀ attention: scores computed transposed [j, i] (keys on partitions) so
    softplus output w feeds (a) out.T = V.T-style matmul lhsT=V[j,d], and
    (b) norm² = ones.T @ w² — both contract over j on partitions.
    softplus = Exp (scale=1/8, PSUM src) then Ln (bias=1.0), fp16 intermediate.
    Causality: block-ragged i-range per j-block + a -1e9 mask matmul on the
    diagonal 128x128 sub-block (softplus underflows to exactly 0).
"""

import numpy as np
import ml_dtypes

import concourse.bacc as bacc
import concourse.tile as tile
import concourse.mybir as mybir
from concourse.bass_utils import run_bass_kernel_spmd

# The act-table-set chooser assigns each activation the FIRST set containing
# its function; with the default ordering Exp -> exp_and_others and
# Ln -> natural_log, so alternating Exp/Ln thrashes ACT_TABLE_LOAD (~1.3us
# each, >100 loads). Reorder so the combined Exp+Ln set is preferred.
_orig_get_tables = bacc.get_activation_tables


def _tables_ln_exp_first(arch):
    t = _orig_get_tables(arch)
    key = "natural_log_exp_and_others"
    if key not in t:
        return t
    # Keep dict ORDER (set ids are positional); drop Exp/Ln from every other
    # set so the combined set is the unique candidate for both.
    exp = mybir.ActivationFunctionType.Exp
    ln = mybir.ActivationFunctionType.Ln
    out = {}
    for k, fns in t.items():
        out[k] = fns if k == key else (set(fns) - {exp, ln})
    return out


bacc.get_activation_tables = _tables_ln_exp_first

dt = mybir.dt
F32, F32R, F16, BF16 = dt.float32, dt.float32r, dt.float16, dt.bfloat16
AF = mybir.ActivationFunctionType

B, T, C, H, HS = 2, 2048, 1024, 16, 64
HPC = 4            # heads per core
NCORES = 8
SCALE = 1.0 / np.sqrt(HS)
TC_Q = 512         # i-chunk (query) width
JB = 128           # j-block (key) width

_CACHE = {}


def _build():
    nc = bacc.Bacc(None, target_bir_lowering=False)

    xT = nc.dram_tensor("xT", [C, T], BF16, kind="ExternalInput")
    wqk = nc.dram_tensor("wqk", [C, 512], BF16, kind="ExternalInput")
    wv = nc.dram_tensor("wv", [C, 256], BF16, kind="ExternalInput")
    wp = nc.dram_tensor("wp", [256, C], BF16, kind="ExternalInput")
    mtri = nc.dram_tensor("mtri", [128, 128], BF16, kind="ExternalInput")
    out = nc.dram_tensor("out", [T, C], F32, kind="ExternalOutput")

    with tile.TileContext(nc) as tc:
        with (
            tc.tile_pool(name="cst", bufs=1) as cst,
            tc.tile_pool(name="data", bufs=1) as data,
        ):
            # ---- constants / weights ----
            wqk_sb = cst.tile([128, 8, 512], BF16)
            nc.sync.dma_start(wqk_sb, wqk[:].rearrange("(po pi) j -> pi po j", pi=128))
            wv_sb = cst.tile([128, 8, 256], BF16)
            nc.sync.dma_start(wv_sb, wv[:].rearrange("(po pi) j -> pi po j", pi=128))
            wp_sb = cst.tile([128, 2, 1024], BF16)
            nc.sync.dma_start(wp_sb, wp[:].rearrange("(po pi) e -> pi po e", pi=128))
            mtri_sb = cst.tile([128, 128], BF16)
            nc.sync.dma_start(mtri_sb, mtri[:])
            ones_n = cst.tile([128, 1], BF16)
            nc.vector.memset(ones_n, 1.0)
            ones_b = cst.tile([128, 64], BF16)
            nc.vector.memset(ones_b, 1.0)

            # ---- x.T resident (4MB bf16), per c-block DMAs so QKV starts early
            xT_sb = data.tile([128, 8, 2048], BF16)
            xT_r = xT[:].rearrange("(po pi) t -> pi po t", pi=128)
            for cb in range(8):
                nc.sync.dma_start(xT_sb[:, cb, :], xT_r[:, cb, :])

            # qkT: block 0,1 = q head-pairs; block 2,3 = k head-pairs.
            # Partition rows (h%2)*64..+64 inside each block = one head.
            qkT = data.tile([128, 4, 2048], BF16)
            v_sb = data.tile([128, 16, 256], BF16)
            yT = data.tile([128, 2, 2048], BF16)

            # ================= phase 1: QKV projections =================
            with tc.tile_pool(name="ps_qkv", bufs=2, space="PSUM") as ps_qkv:
                for jb in range(4):
                    for tcc in range(4):
                        pq = ps_qkv.tile([128, 512], F32, tag="qk")
                        for cb in range(8):
                            nc.tensor.matmul(
                                pq,
                                wqk_sb[:, cb, jb * 128:(jb + 1) * 128],
                                xT_sb[:, cb, tcc * 512:(tcc + 1) * 512],
                                start=(cb == 0), stop=(cb == 7),
                            )
                        nc.vector.tensor_copy(
                            qkT[:, jb, tcc * 512:(tcc + 1) * 512], pq)
                for tb in range(16):
                    pv = ps_qkv.tile([128, 256], F32, tag="v")
                    for cb in range(8):
                        nc.tensor.matmul(
                            pv,
                            xT_sb[:, cb, tb * 128:(tb + 1) * 128],
                            wv_sb[:, cb, :],
                            start=(cb == 0), stop=(cb == 7),
                        )
                    nc.vector.tensor_copy(v_sb[:, tb, :], pv)

            # ================= phase 2: attention (2 head pairs) =================
            with (
                tc.tile_pool(name="ps_s", bufs=2, space="PSUM") as ps_s,   # 4 banks
                tc.tile_pool(name="ps_o", bufs=2, space="PSUM") as ps_o,   # 2 banks
                tc.tile_pool(name="ps_n", bufs=1, space="PSUM") as ps_n,   # 1 bank
                tc.tile_pool(name="ps_b", bufs=1, space="PSUM") as ps_b,   # 1 bank
                tc.tile_pool(name="we", bufs=4) as we,
                tc.tile_pool(name="epi", bufs=3) as epi,
            ):
                import contextlib
                for hp in range(2):
                    qblk, kblk = hp, 2 + hp
                    # hoist pair-0 so its score MMs / exps interleave into the
                    # QKV phase as soon as their qkT/v tiles and PSUM banks free
                    hoist = tc.high_priority() if hp == 0 else contextlib.nullcontext()
                    with hoist:
                      for ic in range(4):
                          po = ps_o.tile([128, 512], F32, tag="po")
                          pn = ps_n.tile([128, 512], F32, tag="pn")
                          njb = 4 * ic + 4
                          for jb in range(njb):
                              m = jb - 4 * ic
                              N = 512 if m < 0 else 512 - 128 * m
                              ioff = ic * 512 + (512 - N)
                              ps_ = ps_s.tile([128, 1024], F32, tag="ps")
                              diag = m >= 0
                              # scores (transposed): K=64, two heads row-packed
                              nc.tensor.matmul(
                                  ps_[:, 0:N],
                                  qkT[0:64, kblk, jb * 128:(jb + 1) * 128],
                                  qkT[0:64, qblk, ioff:ioff + N],
                                  start=True, stop=True,
                              )
                              nc.tensor.matmul(
                                  ps_[:, 512:512 + N],
                                  qkT[64:128, kblk, jb * 128:(jb + 1) * 128],
                                  qkT[64:128, qblk, ioff:ioff + N],
                                  start=True, stop=True,
                              )
                              # softplus = Ln(Exp(s/8) + 1), fp16 intermediate.
                              # A-half at w[:, 0:N], B-half at w[:, N:2N]
                              # (ACT reads the two psum banks via a strided AP).
                              e = we.tile([128, 1024], F16, tag="e")
                              w = we.tile([128, 1024], BF16, tag="w")
                              w2 = we.tile([128, 1024], BF16, tag="w2")
                              if N == 512:
                                  nc.scalar.activation(e, ps_, AF.Exp,
                                                       scale=SCALE)
                              else:
                                  # one strided instr over both psum banks
                                  ps3 = ps_.rearrange(
                                      "p (b n) -> p b n", b=2)[:, :, 0:N]
                                  e3 = e[:, 0:2 * N].rearrange(
                                      "p (b n) -> p b n", b=2)
                                  nc.scalar.activation(e3, ps3, AF.Exp,
                                                       scale=SCALE)
                              nc.scalar.activation(w[:, 0:2 * N], e[:, 0:2 * N],
                                                   AF.Ln, bias=1.0)
                              if diag:
                                  # zero the upper-triangular (j > i) part of the
                                  # leading 128-col diagonal sub-block, both heads
                                  nc.vector.tensor_mul(w[:, 0:128], w[:, 0:128],
                                                       mtri_sb)
                                  nc.vector.tensor_mul(w[:, N:N + 128],
                                                       w[:, N:N + 128], mtri_sb)
                              nc.vector.tensor_mul(w2[:, 0:2 * N], w[:, 0:2 * N],
                                                   w[:, 0:2 * N])
                              # out.T accumulation: lhsT = V[j, d], col-packed heads
                              last = jb == njb - 1
                              hA, hB = 2 * hp, 2 * hp + 1
                              nc.tensor.matmul(
                                  po[0:64, 512 - N:512],
                                  v_sb[:, jb, hA * 64:hA * 64 + 64],
                                  w[:, 0:N],
                                  start=(jb == 0), stop=last, tile_position=(0, 0),
                              )
                              nc.tensor.matmul(
                                  po[64:128, 512 - N:512],
                                  v_sb[:, jb, hB * 64:hB * 64 + 64],
                                  w[:, N:2 * N],
                                  start=(jb == 0), stop=last, tile_position=(0, 64),
                              )
                              # norm^2 accumulation: ones.T @ w2
                              nc.tensor.matmul(
                                  pn[0:1, 512 - N:512], ones_n, w2[:, 0:N],
                                  start=(jb == 0), stop=last, tile_position=(0, 0),
                              )
                              nc.tensor.matmul(
                                  pn[64:65, 512 - N:512], ones_n, w2[:, N:2 * N],
                                  start=(jb == 0), stop=last, tile_position=(0, 64),
                              )
                          # ---- chunk epilogue: y = out.T * rsqrt(norm2) ----
                          # rsqrt = Exp(-0.5 * Ln(x)): stays in the Exp/Ln
                          # activation-table set (no ACT_TABLE_LOAD thrash).
                          # high_priority hoists these ahead of the next chunk's
                          # prefetched exp work in the in-order ACT stream, so
                          # the bcast MM / po release don't stall the PE.
                          with tc.high_priority():
                              nrm = epi.tile([128, 512], F32, tag="nrm")
                              nc.scalar.activation(nrm, pn, AF.Ln)
                              rs = epi.tile([128, 512], BF16, tag="rs")
                              nc.scalar.activation(rs, nrm, AF.Exp, scale=-0.5)
                              pb = ps_b.tile([128, 512], F32, tag="pb")
                              nc.tensor.matmul(pb[0:64, :], ones_b[0:1, :],
                                               rs[0:1, :],
                                               start=True, stop=True,
                                               tile_position=(0, 0))
                              nc.tensor.matmul(pb[64:128, :], ones_b[64:65, :],
                                               rs[64:65, :],
                                               start=True, stop=True,
                                               tile_position=(64, 64))
                              rb = epi.tile([128, 512], F32, tag="rb")
                              nc.vector.tensor_copy(rb, pb)
                              nc.vector.tensor_mul(
                                  yT[:, hp, ic * 512:(ic + 1) * 512], po, rb)

            # ================= phase 3: partial projection =================
            with (
                tc.tile_pool(name="ps_p", bufs=2, space="PSUM") as ps_p,
                tc.tile_pool(name="outp", bufs=3) as outp,
            ):
                for tcc in range(16):
                    pp = ps_p.tile([128, 1024], F32, tag="pp")
                    for kb in range(2):
                        for nk in range(2):
                            nc.tensor.matmul(
                                pp[:, nk * 512:(nk + 1) * 512],
                                yT[:, kb, tcc * 128:(tcc + 1) * 128],
                                wp_sb[:, kb, nk * 512:(nk + 1) * 512],
                                start=(kb == 0), stop=(kb == 1),
                            )
                    os_ = outp.tile([128, 1024], F32, tag="os")
                    nc.vector.tensor_copy(os_, pp)
                    nc.sync.dma_start(out[tcc * 128:(tcc + 1) * 128, :], os_)

    nc.compile()
    return nc


def _prep_inputs(x, W_qkv, W_proj):
    """Host-side shard + layout prep. Returns per-core input maps."""
    bf = ml_dtypes.bfloat16
    mtri = np.triu(np.ones((128, 128), dtype=np.float32)).astype(bf)
    in_maps = []
    for core in range(NCORES):
        b, g = core // 4, core % 4
        heads = range(4 * g, 4 * g + 4)
        # W_qkv rows: q = h*64.., k = C + h*64.., v = 2C + h*64..
        q_rows = np.concatenate([np.arange(h * HS, (h + 1) * HS) for h in heads])
        wqk = np.concatenate(
            [W_qkv[q_rows, :].T, W_qkv[C + q_rows, :].T], axis=1)  # [C, 512]
        wv = W_qkv[2 * C + q_rows, :].T                            # [C, 256]
        wp = W_proj[:, q_rows].T                                   # [256, C]
        in_maps.append({
            "xT": np.ascontiguousarray(x[b].T).astype(bf),
            "wqk": np.ascontiguousarray(wqk).astype(bf),
            "wv": np.ascontiguousarray(wv).astype(bf),
            "wp": np.ascontiguousarray(wp).astype(bf),
            "mtri": mtri,
        })
    return in_maps


def _run(in_maps, trace=False, trace_cores=None):
    if "nc" not in _CACHE:
        _CACHE["nc"] = _build()
    return run_bass_kernel_spmd(
        _CACHE["nc"], in_maps, core_ids=list(range(NCORES)),
        trace=trace, trace_cores=trace_cores,
    )


def kernel(x, W_qkv, W_proj, b_proj):
    x = np.asarray(x, dtype=np.float32)
    W_qkv = np.asarray(W_qkv, dtype=np.float32)
    W_proj = np.asarray(W_proj, dtype=np.float32)
    b_proj = np.asarray(b_proj, dtype=np.float32)

    res = _run(_prep_inputs(x, W_qkv, W_proj)).results
    out = np.zeros((B, T, C), dtype=np.float64)
    for core in range(NCORES):
        out[core // 4] += np.asarray(res[core]["out"], dtype=np.float64)
    out += b_proj.astype(np.float64)
    return out.astype(np.float32)



# revision 3
# speedup vs baseline: 1.1105x; 1.1105x over previous
"""Fused sp2norm-MHA kernel for Trainium2, 8 NeuronCores.

Model (per reference):
    qkv = x @ W_qkv.T ; split heads (H=16, hs=64)
    s = (q @ k.T) / sqrt(hs);  w = softplus(s) causal-masked
    out_h = (w @ v) / ||w||_row ;  out = concat(out_h) @ W_proj.T + b_proj

Sharding: core c = (b, g) with b = c // 4 (batch), g = c % 4 (head group of 4).
Each core computes its batch's QKV for its 4 heads, the attention, and a
partial projection over its 256 feature channels. The host sums the 4 partial
projections per batch and adds the bias (the unshard step).

Schedule (v2): the ACT engine (softplus = Exp then Ln over every causal score
element, ~143us at 1 elem/lane/cycle) is the roofline for this problem, so the
kernel is built around keeping ACT busy from ~13us on:
  - i-chunk-major pipeline: QKV for chunk 0 is produced first, attention for
    chunk ic starts as soon as its q/k slabs exist; QKV for chunk ic+1, the V
    tiles, and the output projection for chunk ic-1 are emitted as "filler"
    PE work interleaved between attention blocks (PE has ~2x slack vs ACT).
  - score/out/norm matmul pairs use disjoint PE row/col tiles (K=64 row-split,
    M=64 / M=1 col-split) so each pair runs concurrently (~1 matmul time).
  - softplus Exp reads score pairs [128, 2N] straight from 2 PSUM banks; for
    full blocks two j-blocks are merged into one Ln over [128, 4N] (fp16
    intermediate) to amortize ACT instruction overhead.
  - norm^2 rows for the two head-pairs land in one PSUM bank at partitions
    {0,64} (pair 0) and {32,96} (pair 1) so one Ln+Exp epilogue per i-chunk
    computes all rsqrt values; out.T is staged unnormalized to SBUF so the
    po PSUM bank recycles immediately, and the rsqrt broadcast (ones-outer-
    product matmul pairs on disjoint tiles) scales it afterwards.
PSUM budget: scores 2x[128,1024] (4 banks) + po (1) + pn (1) + aux ring
(qkv/v/proj/bcast, 2) = 8 banks exactly.
"""

import collections
import numpy as np
import ml_dtypes

import concourse.bacc as bacc
import concourse.tile as tile
import concourse.mybir as mybir
from concourse.bass_utils import run_bass_kernel_spmd

# The act-table-set chooser assigns each activation the FIRST set containing
# its function; with the default ordering Exp -> exp_and_others and
# Ln -> natural_log, so alternating Exp/Ln thrashes ACT_TABLE_LOAD (~1.3us
# each, >100 loads). Reorder so the combined Exp+Ln set is preferred.
_orig_get_tables = bacc.get_activation_tables


def _tables_ln_exp_first(arch):
    t = _orig_get_tables(arch)
    key = "natural_log_exp_and_others"
    if key not in t:
        return t
    # Keep dict ORDER (set ids are positional); drop Exp/Ln from every other
    # set so the combined set is the unique candidate for both.
    exp = mybir.ActivationFunctionType.Exp
    ln = mybir.ActivationFunctionType.Ln
    out = {}
    for k, fns in t.items():
        out[k] = fns if k == key else (set(fns) - {exp, ln})
    return out


bacc.get_activation_tables = _tables_ln_exp_first

dt = mybir.dt
F32, F16, BF16 = dt.float32, dt.float16, dt.bfloat16
AF = mybir.ActivationFunctionType

B, T, C, H, HS = 2, 2048, 1024, 16, 64
NCORES = 8
SCALE = 1.0 / np.sqrt(HS)

_CACHE = {}


def _build():
    nc = bacc.Bacc(None, target_bir_lowering=False)

    xT = nc.dram_tensor("xT", [C, T], BF16, kind="ExternalInput")
    wqk = nc.dram_tensor("wqk", [C, 512], BF16, kind="ExternalInput")
    wv = nc.dram_tensor("wv", [C, 256], BF16, kind="ExternalInput")
    wp = nc.dram_tensor("wp", [256, C], BF16, kind="ExternalInput")
    mtri = nc.dram_tensor("mtri", [128, 128], BF16, kind="ExternalInput")
    out = nc.dram_tensor("out", [T, C], F32, kind="ExternalOutput")

    with tile.TileContext(nc) as tc:
        with (
            tc.tile_pool(name="cst", bufs=1) as cst,
            tc.tile_pool(name="data", bufs=1) as data,
            tc.tile_pool(name="ps", bufs=1, space="PSUM") as ps_pool,
            tc.tile_pool(name="pso", bufs=1, space="PSUM") as pso_pool,
            tc.tile_pool(name="psn", bufs=1, space="PSUM") as psn_pool,
            tc.tile_pool(name="aux", bufs=2, space="PSUM") as aux_pool,
            tc.tile_pool(name="we", bufs=2) as we,
            tc.tile_pool(name="epi", bufs=2) as epi,
            tc.tile_pool(name="outp", bufs=3) as outp,
        ):
            # ---- act-table preload: a tiny Exp at t~0 pulls the single
            # ACT_TABLE_LOAD (natural_log_exp_and_others) off the critical path
            warm = cst.tile([1, 2], F32)
            nc.vector.memset(warm, 0.0)
            warm2 = cst.tile([1, 2], F32)
            nc.scalar.activation(warm2, warm, AF.Exp)

            # ---- constants / weights (DMA issue order == need order) ----
            mtri_sb = cst.tile([128, 128], BF16)
            nc.sync.dma_start(mtri_sb, mtri[:])
            ones_n = cst.tile([128, 1], BF16)
            nc.vector.memset(ones_n, 1.0)
            ones_b = cst.tile([128, 64], BF16)
            nc.vector.memset(ones_b, 1.0)

            wqk_sb = cst.tile([128, 8, 512], BF16)
            wqk_r = wqk[:].rearrange("(po pi) j -> pi po j", pi=128)
            wv_sb = cst.tile([128, 8, 256], BF16)
            wv_r = wv[:].rearrange("(po pi) j -> pi po j", pi=128)
            xT_sb = data.tile([128, 8, 2048], BF16)
            xT_r = xT[:].rearrange("(po pi) t -> pi po t", pi=128)
            # first-needed first: per-cb wqk + xT chunk 0 (scores), then wv
            # (first V tiles), then the remaining xT chunks, wp last.
            for cb in range(8):
                nc.sync.dma_start(wqk_sb[:, cb, :], wqk_r[:, cb, :])
                nc.sync.dma_start(xT_sb[:, cb, 0:512], xT_r[:, cb, 0:512])
            for cb in range(8):
                nc.sync.dma_start(wv_sb[:, cb, :], wv_r[:, cb, :])
            for tq in range(1, 4):
                for cb in range(8):
                    nc.sync.dma_start(xT_sb[:, cb, tq * 512:(tq + 1) * 512],
                                      xT_r[:, cb, tq * 512:(tq + 1) * 512])
            wp_sb = cst.tile([128, 2, 1024], BF16)
            wp_r = wp[:].rearrange("(po pi) e -> pi po e", pi=128)
            for po_ in range(2):
                nc.sync.dma_start(wp_sb[:, po_, :], wp_r[:, po_, :])

            # qkT: block 0,1 = q head-pairs; block 2,3 = k head-pairs.
            # Partition rows (h%2)*64..+64 inside each block = one head.
            qkT = data.tile([128, 4, 2048], BF16)
            v_sb = data.tile([128, 16, 256], BF16)
            yTu = data.tile([128, 2, 2048], BF16)   # unnormalized out.T
            yT = data.tile([128, 2, 2048], BF16)    # normalized out.T

            # scores psum: one 4-bank tile, manually split in two
            # double-buffered halves (range-level deps track the halves).
            ps_all = ps_pool.tile([128, 2048], F32)

            # ---------- PE filler units (QKV slabs / V tiles / proj) -------
            def qk_slab(jb, tcc):
                pq = aux_pool.tile([128, 512], F32, tag="aux")
                for cb in range(8):
                    nc.tensor.matmul(
                        pq,
                        wqk_sb[:, cb, jb * 128:(jb + 1) * 128],
                        xT_sb[:, cb, tcc * 512:(tcc + 1) * 512],
                        start=(cb == 0), stop=(cb == 7),
                    )
                nc.vector.tensor_copy(
                    qkT[:, jb, tcc * 512:(tcc + 1) * 512], pq)

            def v_tile(tb):
                pv = aux_pool.tile([128, 512], F32, tag="aux")
                for cb in range(8):
                    nc.tensor.matmul(
                        pv[:, 0:256],
                        xT_sb[:, cb, tb * 128:(tb + 1) * 128],
                        wv_sb[:, cb, :],
                        start=(cb == 0), stop=(cb == 7),
                    )
                nc.vector.tensor_copy(v_sb[:, tb, :], pv[:, 0:256])

            def proj_half(tcc, nk):
                pp = aux_pool.tile([128, 512], F32, tag="aux")
                for kb in range(2):
                    nc.tensor.matmul(
                        pp,
                        yT[:, kb, tcc * 128:(tcc + 1) * 128],
                        wp_sb[:, kb, nk * 512:(nk + 1) * 512],
                        start=(kb == 0), stop=(kb == 1),
                    )
                os_ = outp.tile([128, 512], F32, tag="os")
                nc.vector.tensor_copy(os_, pp)
                nc.sync.dma_start(
                    out[tcc * 128:(tcc + 1) * 128, nk * 512:(nk + 1) * 512],
                    os_)

            fillers = collections.deque()

            def pop_fillers(n):
                for _ in range(n):
                    if not fillers:
                        return
                    fillers.popleft()()

            # ---------------- attention building blocks -------------------
            def scores_block(hp, ic, jb, half):
                """Score pair for j-block jb against i-chunk ic; returns
                (ps half AP, N)."""
                qblk, kblk = hp, 2 + hp
                m = jb - 4 * ic
                N = 512 if m < 0 else 512 - 128 * m
                ioff = ic * 512 + (512 - N)
                ps_ = ps_all[:, half * 1024:half * 1024 + 1024]
                nc.tensor.matmul(
                    ps_[:, 0:N],
                    qkT[0:64, kblk, jb * 128:(jb + 1) * 128],
                    qkT[0:64, qblk, ioff:ioff + N],
                    start=True, stop=True,
                )
                nc.tensor.matmul(
                    ps_[:, 512:512 + N],
                    qkT[64:128, kblk, jb * 128:(jb + 1) * 128],
                    qkT[64:128, qblk, ioff:ioff + N],
                    start=True, stop=True,
                )
                return ps_, N

            def softplus_single(ps_, N, diag):
                """softplus = Ln(Exp(s/8) + 1), fp16 intermediate; one block.
                A-half at w[:, 0:N], B-half at w[:, N:2N]."""
                e = we.tile([128, 1024], F16, tag="e1")
                w = we.tile([128, 1024], BF16, tag="w1")
                w2 = we.tile([128, 1024], BF16, tag="ww1")
                if N == 512:
                    nc.scalar.activation(e, ps_, AF.Exp, scale=SCALE)
                else:
                    ps3 = ps_.rearrange("p (b n) -> p b n", b=2)[:, :, 0:N]
                    e3 = e[:, 0:2 * N].rearrange("p (b n) -> p b n", b=2)
                    nc.scalar.activation(e3, ps3, AF.Exp, scale=SCALE)
                nc.scalar.activation(w[:, 0:2 * N], e[:, 0:2 * N],
                                     AF.Ln, bias=1.0)
                if diag:
                    nc.vector.tensor_mul(w[:, 0:128], w[:, 0:128], mtri_sb)
                    nc.vector.tensor_mul(w[:, N:N + 128], w[:, N:N + 128],
                                         mtri_sb)
                nc.vector.tensor_mul(w2[:, 0:2 * N], w[:, 0:2 * N],
                                     w[:, 0:2 * N])
                return w, w2

            def out_norm_block(hp, ic, jb, w, w2, woff, po, pn, start, stop):
                """out.T and norm^2 accumulation for one block; w/w2 tile
                slices start at free offset woff, layout [A(N) | B(N)]."""
                m = jb - 4 * ic
                N = 512 if m < 0 else 512 - 128 * m
                hA, hB = 2 * hp, 2 * hp + 1
                nc.tensor.matmul(
                    po[0:64, 512 - N:512],
                    v_sb[:, jb, hA * 64:hA * 64 + 64],
                    w[:, woff:woff + N],
                    start=start, stop=stop, tile_position=(0, 0),
                )
                nc.tensor.matmul(
                    po[64:128, 512 - N:512],
                    v_sb[:, jb, hB * 64:hB * 64 + 64],
                    w[:, woff + N:woff + 2 * N],
                    start=start, stop=stop, tile_position=(0, 64),
                )
                rA, rB = 32 * hp, 64 + 32 * hp
                nc.tensor.matmul(
                    pn[rA:rA + 1, 512 - N:512], ones_n,
                    w2[:, woff:woff + N],
                    start=start, stop=stop, tile_position=(0, rA),
                )
                nc.tensor.matmul(
                    pn[rB:rB + 1, 512 - N:512], ones_n,
                    w2[:, woff + N:woff + 2 * N],
                    start=start, stop=stop, tile_position=(0, rB),
                )

            # ------------------------- pipeline ---------------------------
            # prologue: q/k slabs + v tiles for chunk 0
            qk_slab(0, 0)   # q pair 0
            qk_slab(2, 0)   # k pair 0
            v_tile(0)
            v_tile(1)
            v_tile(2)
            v_tile(3)
            fillers.append(lambda: qk_slab(1, 0))   # q pair 1
            fillers.append(lambda: qk_slab(3, 0))   # k pair 1

            flip = 0
            for ic in range(4):
                if ic < 3:
                    tn = ic + 1
                    for jb_ in (0, 2, 1, 3):
                        fillers.append(
                            lambda jb_=jb_, tn=tn: qk_slab(jb_, tn))
                    for tb_ in range(4 * tn, 4 * tn + 4):
                        fillers.append(lambda tb_=tb_: v_tile(tb_))
                if ic > 0:
                    for tcc_ in range(4 * (ic - 1), 4 * (ic - 1) + 4):
                        for nk_ in range(2):
                            fillers.append(
                                lambda tcc_=tcc_, nk_=nk_:
                                proj_half(tcc_, nk_))

                njb = 4 * ic + 4
                pn = psn_pool.tile([128, 512], F32, tag="pn")

                # software pipeline (lag 1): emit unit k+1's scores+softplus
                # BEFORE unit k's out/norm matmuls, so the PE stream never
                # parks on Ln(k) with ACT's next input not yet computed.
                def emit_out_norm(pend):
                    hp_, jbs_, w_, w2_, po_ = pend
                    for i_, jb_ in enumerate(jbs_):
                        out_norm_block(hp_, ic, jb_, w_, w2_, 1024 * i_,
                                       po_, pn, jb_ == 0,
                                       jb_ == njb - 1)
                    if jbs_[-1] == njb - 1:
                        # stage unnormalized out.T; frees po for next pair
                        nc.vector.tensor_copy(
                            yTu[:, hp_, ic * 512:(ic + 1) * 512], po_)

                pending = None
                for hp in range(2):
                    po = pso_pool.tile([128, 512], F32, tag="po")
                    jb = 0
                    while jb < njb:
                        if jb + 1 < 4 * ic:  # merge two full blocks
                            psA, _ = scores_block(hp, ic, jb, 0)
                            psB, _ = scores_block(hp, ic, jb + 1, 1)
                            # one Exp per psum bank-pair, one Ln over both
                            e = we.tile([128, 2048], F16, tag="e2")
                            w = we.tile([128, 2048], BF16, tag="w2")
                            w2 = we.tile([128, 2048], BF16, tag="ww2")
                            nc.scalar.activation(e[:, 0:1024], psA, AF.Exp,
                                                 scale=SCALE)
                            nc.scalar.activation(e[:, 1024:2048], psB, AF.Exp,
                                                 scale=SCALE)
                            nc.scalar.activation(w, e, AF.Ln, bias=1.0)
                            nc.vector.tensor_mul(w2, w, w)
                            cur = (hp, (jb, jb + 1), w, w2, po)
                            jb += 2
                        else:
                            m = jb - 4 * ic
                            ps_, N = scores_block(hp, ic, jb, flip)
                            flip ^= 1
                            w, w2 = softplus_single(ps_, N, m >= 0)
                            cur = (hp, (jb,), w, w2, po)
                            jb += 1
                        pop_fillers(1)
                        if pending is not None:
                            emit_out_norm(pending)
                            pop_fillers(1)
                        pending = cur
                emit_out_norm(pending)
                pending = None

                # ---- chunk epilogue: rsqrt(norm2) for all 4 heads ----
                # rsqrt = Exp(-0.5 * Ln(x)): stays in the Exp/Ln set.
                # Rows {0,64} = pair 0, {32,96} = pair 1; unread rows compute
                # garbage harmlessly (ACT cost is free-dim bound).
                nrm = epi.tile([128, 512], F32, tag="nrm")
                nc.scalar.activation(nrm, pn, AF.Ln)
                rs = epi.tile([128, 512], BF16, tag="rs")
                nc.scalar.activation(rs, nrm, AF.Exp, scale=-0.5)
                for hp in range(2):
                    rA, rB = 32 * hp, 64 + 32 * hp
                    pb = aux_pool.tile([128, 512], F32, tag="aux")
                    nc.tensor.matmul(pb[0:64, :], ones_b[rA:rA + 1, :],
                                     rs[rA:rA + 1, :],
                                     start=True, stop=True,
                                     tile_position=(rA, 0))
                    nc.tensor.matmul(pb[64:128, :], ones_b[rB:rB + 1, :],
                                     rs[rB:rB + 1, :],
                                     start=True, stop=True,
                                     tile_position=(rB, 64))
                    rb = epi.tile([128, 512], F32, tag="rb")
                    nc.vector.tensor_copy(rb, pb)
                    nc.vector.tensor_mul(
                        yT[:, hp, ic * 512:(ic + 1) * 512],
                        yTu[:, hp, ic * 512:(ic + 1) * 512], rb)

                # anything not yet emitted must land before the next chunk
                pop_fillers(len(fillers))

            # last chunk's projection
            for tcc_ in range(12, 16):
                for nk_ in range(2):
                    proj_half(tcc_, nk_)

    nc.compile()
    return nc


def _prep_inputs(x, W_qkv, W_proj):
    """Host-side shard + layout prep. Returns per-core input maps."""
    bf = ml_dtypes.bfloat16
    mtri = np.triu(np.ones((128, 128), dtype=np.float32)).astype(bf)
    in_maps = []
    for core in range(NCORES):
        b, g = core // 4, core % 4
        heads = range(4 * g, 4 * g + 4)
        # W_qkv rows: q = h*64.., k = C + h*64.., v = 2C + h*64..
        q_rows = np.concatenate([np.arange(h * HS, (h + 1) * HS) for h in heads])
        wqk = np.concatenate(
            [W_qkv[q_rows, :].T, W_qkv[C + q_rows, :].T], axis=1)  # [C, 512]
        wv = W_qkv[2 * C + q_rows, :].T                            # [C, 256]
        wp = W_proj[:, q_rows].T                                   # [256, C]
        in_maps.append({
            "xT": np.ascontiguousarray(x[b].T).astype(bf),
            "wqk": np.ascontiguousarray(wqk).astype(bf),
            "wv": np.ascontiguousarray(wv).astype(bf),
            "wp": np.ascontiguousarray(wp).astype(bf),
            "mtri": mtri,
        })
    return in_maps


def _run(in_maps, trace=False, trace_cores=None):
    if "nc" not in _CACHE:
        _CACHE["nc"] = _build()
    return run_bass_kernel_spmd(
        _CACHE["nc"], in_maps, core_ids=list(range(NCORES)),
        trace=trace, trace_cores=trace_cores,
    )


def kernel(x, W_qkv, W_proj, b_proj):
    x = np.asarray(x, dtype=np.float32)
    W_qkv = np.asarray(W_qkv, dtype=np.float32)
    W_proj = np.asarray(W_proj, dtype=np.float32)
    b_proj = np.asarray(b_proj, dtype=np.float32)

    res = _run(_prep_inputs(x, W_qkv, W_proj)).results
    out = np.zeros((B, T, C), dtype=np.float64)
    for core in range(NCORES):
        out[core // 4] += np.asarray(res[core]["out"], dtype=np.float64)
    out += b_proj.astype(np.float64)
    return out.astype(np.float32)


# revision 4
# speedup vs baseline: 1.1827x; 1.0650x over previous
"""Fused sp2norm-MHA kernel for Trainium2, 8 NeuronCores.

Model (per reference):
    qkv = x @ W_qkv.T ; split heads (H=16, hs=64)
    s = (q @ k.T) / sqrt(hs);  w = softplus(s) causal-masked
    out_h = (w @ v) / ||w||_row ;  out = concat(out_h) @ W_proj.T + b_proj

Sharding: core c = (b, g) with b = c // 4 (batch), g = c % 4 (head group of 4).
Each core computes its batch's QKV for its 4 heads, the attention, and a
partial projection over its 256 feature channels. The host sums the 4 partial
projections per batch and adds the bias (the unshard step).

Schedule: the ACT engine (softplus = Exp then Ln over every causal score
element, ~143us at 1 elem/lane/cycle) is the roofline for this problem, so the
kernel is built around keeping ACT busy from ~13us on:
  - i-chunk-major pipeline: attention for chunk ic starts as soon as its q/k
    slabs exist; QKV for later chunks, the V tiles, and the output projection
    are emitted as "filler" PE work interleaved between attention blocks,
    paced adaptively and deferred toward the late (ACT-rich) chunks.
  - score/out/norm matmul pairs use disjoint PE row/col tiles (K=64 row-split,
    M=64 / M=1 col-split) so each pair runs concurrently (~1 matmul time).
  - lag-1 software pipeline: unit k+1's scores+softplus are emitted before
    unit k's out/norm matmuls so the PE stream never parks on Ln(k) with
    ACT's next input missing.
  - softplus Exp reads score pairs [128, 2N] straight from 2 PSUM banks; for
    full blocks two j-blocks share one Ln over [128, 4N] (fp16 intermediate)
    to amortize ACT instruction overhead.
  - norm^2 rows for the two head-pairs land in one PSUM bank at partitions
    {0,64} (pair 0) and {32,96} (pair 1); one Ln+Exp epilogue per i-chunk
    computes every rsqrt; out.T is staged unnormalized to SBUF so the po
    PSUM bank recycles immediately.
  - host pre-lays inputs out so every input DMA moves >=2KB contiguous per
    partition line (DMA granularity sweet spot).
PSUM budget: scores [128,2048] (4 banks) + po (1) + pn (1) + aux ring
(qkv/v/proj/bcast, 2) = 8 banks exactly.
"""

import collections
import numpy as np
import ml_dtypes

import concourse.bacc as bacc
import concourse.tile as tile
import concourse.mybir as mybir
from concourse.bass_utils import run_bass_kernel_spmd

# The act-table-set chooser assigns each activation the FIRST set containing
# its function; with the default ordering Exp -> exp_and_others and
# Ln -> natural_log, so alternating Exp/Ln thrashes ACT_TABLE_LOAD (~1.3us
# each, >100 loads). Reorder so the combined Exp+Ln set is preferred.
_orig_get_tables = bacc.get_activation_tables


def _tables_ln_exp_first(arch):
    t = _orig_get_tables(arch)
    key = "natural_log_exp_and_others"
    if key not in t:
        return t
    # Keep dict ORDER (set ids are positional); drop Exp/Ln from every other
    # set so the combined set is the unique candidate for both.
    exp = mybir.ActivationFunctionType.Exp
    ln = mybir.ActivationFunctionType.Ln
    out = {}
    for k, fns in t.items():
        out[k] = fns if k == key else (set(fns) - {exp, ln})
    return out


bacc.get_activation_tables = _tables_ln_exp_first

dt = mybir.dt
F32, F16, BF16 = dt.float32, dt.float16, dt.bfloat16
AF = mybir.ActivationFunctionType

B, T, C, H, HS = 2, 2048, 1024, 16, 64
NCORES = 8
SCALE = 1.0 / np.sqrt(HS)

_CACHE = {}


def _build():
    nc = bacc.Bacc(None, target_bir_lowering=False)

    # host-prearranged layouts: partition dim first, >=2KB contiguous lines
    xT = nc.dram_tensor("xT", [128, 4, 4, 1024], BF16, kind="ExternalInput")
    wqk = nc.dram_tensor("wqk", [128, 4, 1024], BF16, kind="ExternalInput")
    wv = nc.dram_tensor("wv", [128, 2, 1024], BF16, kind="ExternalInput")
    wp = nc.dram_tensor("wp", [128, 2, 1024], BF16, kind="ExternalInput")
    mtri = nc.dram_tensor("mtri", [128, 128], BF16, kind="ExternalInput")
    out = nc.dram_tensor("out", [T, C], F32, kind="ExternalOutput")

    with tile.TileContext(nc) as tc:
        with (
            tc.tile_pool(name="cst", bufs=1) as cst,
            tc.tile_pool(name="data", bufs=1) as data,
            tc.tile_pool(name="ps", bufs=1, space="PSUM") as ps_pool,
            tc.tile_pool(name="pso", bufs=1, space="PSUM") as pso_pool,
            tc.tile_pool(name="psn", bufs=1, space="PSUM") as psn_pool,
            tc.tile_pool(name="aux", bufs=2, space="PSUM") as aux_pool,
            tc.tile_pool(name="we", bufs=2) as we,
            tc.tile_pool(name="epi", bufs=2) as epi,
            tc.tile_pool(name="outp", bufs=3) as outp,
        ):
            # ---- act-table preload: a tiny Exp at t~0 pulls the single
            # ACT_TABLE_LOAD (natural_log_exp_and_others) off the critical path
            warm = cst.tile([1, 2], F32)
            nc.vector.memset(warm, 0.0)
            warm2 = cst.tile([1, 2], F32)
            nc.scalar.activation(warm2, warm, AF.Exp)

            # ---- constants / weights (DMA issue order == need order) ----
            mtri_sb = cst.tile([128, 128], BF16)
            nc.sync.dma_start(mtri_sb, mtri[:])
            ones_n = cst.tile([128, 1], BF16)
            nc.vector.memset(ones_n, 1.0)
            ones_b = cst.tile([128, 64], BF16)
            nc.vector.memset(ones_b, 1.0)

            # wqk_sb[:, cp, c2*512 + jb*128 + j] = W_qkv row (2cp+c2)*128+pi
            wqk_sb = cst.tile([128, 4, 1024], BF16)
            # xT_sb[:, tq, cp, c2*512 + tj] = x.T[(2cp+c2)*128+pi, tq*512+tj]
            xT_sb = data.tile([128, 4, 4, 1024], BF16)
            wv_sb = cst.tile([128, 2, 1024], BF16)
            wp_sb = cst.tile([128, 2, 1024], BF16)
            for cp in range(4):
                nc.sync.dma_start(wqk_sb[:, cp, :], wqk[:, cp, :])
            for cp in range(4):
                nc.sync.dma_start(xT_sb[:, 0, cp, :], xT[:, 0, cp, :])
            for cq in range(2):
                nc.sync.dma_start(wv_sb[:, cq, :], wv[:, cq, :])
            for tq in range(1, 4):
                for cp in range(4):
                    nc.sync.dma_start(xT_sb[:, tq, cp, :], xT[:, tq, cp, :])
            for po_ in range(2):
                nc.sync.dma_start(wp_sb[:, po_, :], wp[:, po_, :])

            def x_mv(cb, tcc):     # moving [128, 512] for q/k slab
                return xT_sb[:, tcc, cb // 2,
                             (cb % 2) * 512:(cb % 2) * 512 + 512]

            def x_st(cb, tb):      # stationary [128, 128] for v tile
                off = (cb % 2) * 512 + (tb % 4) * 128
                return xT_sb[:, tb // 4, cb // 2, off:off + 128]

            # qkT: block 0,1 = q head-pairs; block 2,3 = k head-pairs.
            # Partition rows (h%2)*64..+64 inside each block = one head.
            qkT = data.tile([128, 4, 2048], BF16)
            v_sb = data.tile([128, 16, 256], BF16)
            yTu = data.tile([128, 2, 2048], BF16)   # unnormalized out.T
            yT = data.tile([128, 2, 2048], BF16)    # normalized out.T

            # scores psum: one 4-bank tile, manually split in two
            # double-buffered halves (range-level deps track the halves).
            ps_all = ps_pool.tile([128, 2048], F32)

            # ---------- PE filler units (QKV slabs / V tiles / proj) -------
            def qk_slab(jb, tcc):
                pq = aux_pool.tile([128, 512], F32, tag="aux")
                for cb in range(8):
                    nc.tensor.matmul(
                        pq,
                        wqk_sb[:, cb // 2,
                               (cb % 2) * 512 + jb * 128:
                               (cb % 2) * 512 + jb * 128 + 128],
                        x_mv(cb, tcc),
                        start=(cb == 0), stop=(cb == 7),
                    )
                nc.vector.tensor_copy(
                    qkT[:, jb, tcc * 512:(tcc + 1) * 512], pq)

            def v_tile(tb):
                pv = aux_pool.tile([128, 512], F32, tag="aux")
                for cb in range(8):
                    nc.tensor.matmul(
                        pv[:, 0:256],
                        x_st(cb, tb),
                        wv_sb[:, cb // 4,
                              (cb % 4) * 256:(cb % 4) * 256 + 256],
                        start=(cb == 0), stop=(cb == 7),
                    )
                nc.vector.tensor_copy(v_sb[:, tb, :], pv[:, 0:256])

            def proj_half(tcc, nk):
                pp = aux_pool.tile([128, 512], F32, tag="aux")
                for kb in range(2):
                    nc.tensor.matmul(
                        pp,
                        yT[:, kb, tcc * 128:(tcc + 1) * 128],
                        wp_sb[:, kb, nk * 512:(nk + 1) * 512],
                        start=(kb == 0), stop=(kb == 1),
                    )
                os_ = outp.tile([128, 512], F32, tag="os")
                nc.vector.tensor_copy(os_, pp)
                nc.sync.dma_start(
                    out[tcc * 128:(tcc + 1) * 128, nk * 512:(nk + 1) * 512],
                    os_)

            fillers = collections.deque()

            def pop_fillers(n):
                for _ in range(n):
                    if not fillers:
                        return
                    fillers.popleft()()

            # ---------------- attention building blocks -------------------
            def scores_block(hp, ic, jb, half):
                """Score pair for j-block jb against i-chunk ic; returns
                (ps half AP, N)."""
                qblk, kblk = hp, 2 + hp
                m = jb - 4 * ic
                N = 512 if m < 0 else 512 - 128 * m
                ioff = ic * 512 + (512 - N)
                ps_ = ps_all[:, half * 1024:half * 1024 + 1024]
                nc.tensor.matmul(
                    ps_[:, 0:N],
                    qkT[0:64, kblk, jb * 128:(jb + 1) * 128],
                    qkT[0:64, qblk, ioff:ioff + N],
                    start=True, stop=True,
                )
                nc.tensor.matmul(
                    ps_[:, 512:512 + N],
                    qkT[64:128, kblk, jb * 128:(jb + 1) * 128],
                    qkT[64:128, qblk, ioff:ioff + N],
                    start=True, stop=True,
                )
                return ps_, N

            def softplus_single(ps_, N, diag):
                """softplus = Ln(Exp(s/8) + 1), fp16 intermediate; one block.
                A-half at w[:, 0:N], B-half at w[:, N:2N]."""
                e = we.tile([128, 1024], F16, tag="e1")
                w = we.tile([128, 1024], BF16, tag="w1")
                w2 = we.tile([128, 1024], BF16, tag="ww1")
                if N == 512:
                    nc.scalar.activation(e, ps_, AF.Exp, scale=SCALE)
                else:
                    ps3 = ps_.rearrange("p (b n) -> p b n", b=2)[:, :, 0:N]
                    e3 = e[:, 0:2 * N].rearrange("p (b n) -> p b n", b=2)
                    nc.scalar.activation(e3, ps3, AF.Exp, scale=SCALE)
                nc.scalar.activation(w[:, 0:2 * N], e[:, 0:2 * N],
                                     AF.Ln, bias=1.0)
                if diag:
                    nc.vector.tensor_mul(w[:, 0:128], w[:, 0:128], mtri_sb)
                    nc.vector.tensor_mul(w[:, N:N + 128], w[:, N:N + 128],
                                         mtri_sb)
                nc.vector.tensor_mul(w2[:, 0:2 * N], w[:, 0:2 * N],
                                     w[:, 0:2 * N])
                return w, w2

            def out_norm_block(hp, ic, njb, jb, w, w2, woff, po, pn):
                """out.T and norm^2 accumulation for one block; w/w2 tile
                slices start at free offset woff, layout [A(N) | B(N)]."""
                m = jb - 4 * ic
                N = 512 if m < 0 else 512 - 128 * m
                start, stop = jb == 0, jb == njb - 1
                hA, hB = 2 * hp, 2 * hp + 1
                nc.tensor.matmul(
                    po[0:64, 512 - N:512],
                    v_sb[:, jb, hA * 64:hA * 64 + 64],
                    w[:, woff:woff + N],
                    start=start, stop=stop, tile_position=(0, 0),
                )
                nc.tensor.matmul(
                    po[64:128, 512 - N:512],
                    v_sb[:, jb, hB * 64:hB * 64 + 64],
                    w[:, woff + N:woff + 2 * N],
                    start=start, stop=stop, tile_position=(0, 64),
                )
                rA, rB = 32 * hp, 64 + 32 * hp
                nc.tensor.matmul(
                    pn[rA:rA + 1, 512 - N:512], ones_n,
                    w2[:, woff:woff + N],
                    start=start, stop=stop, tile_position=(0, rA),
                )
                nc.tensor.matmul(
                    pn[rB:rB + 1, 512 - N:512], ones_n,
                    w2[:, woff + N:woff + 2 * N],
                    start=start, stop=stop, tile_position=(0, rB),
                )

            # ------------------------- pipeline ---------------------------
            # prologue: only what the first score blocks need
            qk_slab(0, 0)   # q pair 0
            qk_slab(2, 0)   # k pair 0
            for tb_ in range(4):
                fillers.append(lambda tb_=tb_: v_tile(tb_))
            fillers.append(lambda: qk_slab(1, 0))   # q pair 1
            fillers.append(lambda: qk_slab(3, 0))   # k pair 1

            flip = 0
            for ic in range(4):
                # queue this chunk's filler PE work; projection is deferred
                # toward the late, ACT-rich chunks.
                if ic < 3:
                    tn = ic + 1
                    for jb_ in (0, 2, 1, 3):
                        fillers.append(
                            lambda jb_=jb_, tn=tn: qk_slab(jb_, tn))
                    for tb_ in range(4 * tn, 4 * tn + 4):
                        fillers.append(lambda tb_=tb_: v_tile(tb_))
                for pic in ((0,) if ic == 2 else (1, 2) if ic == 3 else ()):
                    for tcc_ in range(4 * pic, 4 * pic + 4):
                        for nk_ in range(2):
                            fillers.append(
                                lambda tcc_=tcc_, nk_=nk_:
                                proj_half(tcc_, nk_))

                njb = 4 * ic + 4
                units_left = 2 * (2 * ic + 4)
                pn = psn_pool.tile([128, 512], F32, tag="pn")

                # lag-1 software pipeline: emit unit k+1's scores+softplus
                # BEFORE unit k's out/norm matmuls.
                def emit_out_norm(pend):
                    hp_, jbs_, w_, w2_, po_ = pend
                    for i_, jb_ in enumerate(jbs_):
                        out_norm_block(hp_, ic, njb, jb_, w_, w2_, 1024 * i_,
                                       po_, pn)
                    if jbs_[-1] == njb - 1:
                        # stage unnormalized out.T; frees po for next pair
                        nc.vector.tensor_copy(
                            yTu[:, hp_, ic * 512:(ic + 1) * 512], po_)

                pending = None
                for hp in range(2):
                    po = pso_pool.tile([128, 512], F32, tag="po")
                    jb = 0
                    while jb < njb:
                        if jb + 1 < 4 * ic:  # merge two full blocks
                            psA, _ = scores_block(hp, ic, jb, 0)
                            psB, _ = scores_block(hp, ic, jb + 1, 1)
                            # one Exp per psum bank-pair, one Ln over both
                            e = we.tile([128, 2048], F16, tag="e2")
                            w = we.tile([128, 2048], BF16, tag="w2")
                            w2 = we.tile([128, 2048], BF16, tag="ww2")
                            nc.scalar.activation(e[:, 0:1024], psA, AF.Exp,
                                                 scale=SCALE)
                            nc.scalar.activation(e[:, 1024:2048], psB, AF.Exp,
                                                 scale=SCALE)
                            nc.scalar.activation(w, e, AF.Ln, bias=1.0)
                            nc.vector.tensor_mul(w2, w, w)
                            cur = (hp, (jb, jb + 1), w, w2, po)
                            jb += 2
                        else:
                            m = jb - 4 * ic
                            ps_, N = scores_block(hp, ic, jb, flip)
                            flip ^= 1
                            w, w2 = softplus_single(ps_, N, m >= 0)
                            cur = (hp, (jb,), w, w2, po)
                            jb += 1
                        # adaptive pacing: leave no backlog for chunk end
                        quota = max(1, -(-len(fillers) // max(units_left, 1)))
                        pop_fillers(min(quota, 2))
                        if pending is not None:
                            emit_out_norm(pending)
                            pop_fillers(1 if quota > 2 else 0)
                        pending = cur
                        units_left -= 1
                emit_out_norm(pending)
                pending = None

                # ---- chunk epilogue: rsqrt(norm2) for all 4 heads ----
                # rsqrt = Exp(-0.5 * Ln(x)): stays in the Exp/Ln set.
                # Rows {0,64} = pair 0, {32,96} = pair 1; unread rows compute
                # garbage harmlessly (ACT cost is free-dim bound).
                nrm = epi.tile([128, 512], F32, tag="nrm")
                nc.scalar.activation(nrm, pn, AF.Ln)
                rs = epi.tile([128, 512], BF16, tag="rs")
                nc.scalar.activation(rs, nrm, AF.Exp, scale=-0.5)
                for hp in range(2):
                    rA, rB = 32 * hp, 64 + 32 * hp
                    pb = aux_pool.tile([128, 512], F32, tag="aux")
                    nc.tensor.matmul(pb[0:64, :], ones_b[rA:rA + 1, :],
                                     rs[rA:rA + 1, :],
                                     start=True, stop=True,
                                     tile_position=(rA, 0))
                    nc.tensor.matmul(pb[64:128, :], ones_b[rB:rB + 1, :],
                                     rs[rB:rB + 1, :],
                                     start=True, stop=True,
                                     tile_position=(rB, 64))
                    rb = epi.tile([128, 512], F32, tag="rb")
                    nc.vector.tensor_copy(rb, pb)
                    nc.vector.tensor_mul(
                        yT[:, hp, ic * 512:(ic + 1) * 512],
                        yTu[:, hp, ic * 512:(ic + 1) * 512], rb)

                # anything not yet emitted must land before the next chunk
                pop_fillers(len(fillers))

            # last chunk's projection
            for tcc_ in range(12, 16):
                for nk_ in range(2):
                    proj_half(tcc_, nk_)

    nc.compile()
    return nc


def _prep_inputs(x, W_qkv, W_proj):
    """Host-side shard + layout prep. Returns per-core input maps.

    Layouts are DMA-friendly: partition index first, then chunk indices so
    every dma_start moves a contiguous >=2KB line per partition.
    """
    bf = ml_dtypes.bfloat16
    mtri = np.triu(np.ones((128, 128), dtype=np.float32)).astype(bf)
    in_maps = []
    for core in range(NCORES):
        b, g = core // 4, core % 4
        heads = range(4 * g, 4 * g + 4)
        # W_qkv rows: q = h*64.., k = C + h*64.., v = 2C + h*64..
        q_rows = np.concatenate([np.arange(h * HS, (h + 1) * HS) for h in heads])
        wqk = np.concatenate(
            [W_qkv[q_rows, :].T, W_qkv[C + q_rows, :].T], axis=1)  # [C, 512]
        wv = W_qkv[2 * C + q_rows, :].T                            # [C, 256]
        wp = W_proj[:, q_rows].T                                   # [256, C]

        xT_ = np.ascontiguousarray(x[b].T).astype(bf)              # [C, T]
        # [128, tq, cp, c2*512+tj]
        xh = xT_.reshape(4, 2, 128, 4, 512).transpose(2, 3, 0, 1, 4)
        xh = np.ascontiguousarray(xh).reshape(128, 4, 4, 1024)
        # [128, cp, c2*512+j]
        wqkh = wqk.astype(bf).reshape(4, 2, 128, 512).transpose(2, 0, 1, 3)
        wqkh = np.ascontiguousarray(wqkh).reshape(128, 4, 1024)
        # [128, cq, c4*256+j]
        wvh = wv.astype(bf).reshape(2, 4, 128, 256).transpose(2, 0, 1, 3)
        wvh = np.ascontiguousarray(wvh).reshape(128, 2, 1024)
        # [128, po, e]
        wph = np.ascontiguousarray(
            wp.astype(bf).reshape(2, 128, 1024).transpose(1, 0, 2))

        in_maps.append({
            "xT": xh, "wqk": wqkh, "wv": wvh, "wp": wph, "mtri": mtri,
        })
    return in_maps


def _run(in_maps, trace=False, trace_cores=None):
    if "nc" not in _CACHE:
        _CACHE["nc"] = _build()
    return run_bass_kernel_spmd(
        _CACHE["nc"], in_maps, core_ids=list(range(NCORES)),
        trace=trace, trace_cores=trace_cores,
    )


def kernel(x, W_qkv, W_proj, b_proj):
    x = np.asarray(x, dtype=np.float32)
    W_qkv = np.asarray(W_qkv, dtype=np.float32)
    W_proj = np.asarray(W_proj, dtype=np.float32)
    b_proj = np.asarray(b_proj, dtype=np.float32)

    res = _run(_prep_inputs(x, W_qkv, W_proj)).results
    out = np.zeros((B, T, C), dtype=np.float64)
    for core in range(NCORES):
        out[core // 4] += np.asarray(res[core]["out"], dtype=np.float64)
    out += b_proj.astype(np.float64)
    return out.astype(np.float32)
